# revision 1
# baseline (speedup 1.0000x reference)
"""Distributed causal attention kernel for one TRN2 chip (8 NeuronCores).

Reference (N=8192, D=1024, fp32):
    qkv = x @ Wqkv; q,k,v = split(qkv)
    sim = (q @ k.T)/sqrt(D) causal-masked; out = softmax(sim) @ v @ Wout + bout

Sharding: CYCLIC sequence-parallel.  Core c owns rows {c, c+8, ...} (1024
rows, indexed m = row//8).  Cyclic sharding makes the causal block
structure IDENTICAL on every core (required: run_bass_kernel_spmd runs
one SPMD graph on all 8 cores) and balances causal work perfectly.  The
+-7 row offset between cores is handled by per-core triangular mask
tiles passed as data (input "msk"), not baked into the graph.

Key algebraic trick: (P @ V) @ Wout = P @ (V @ Wout).  Each core folds
Wout into its own V shard BEFORE the AllGather (V' = V @ Wout, same
FLOPs as the output projection it replaces), so the PV matmul directly
produces the final output in natural [row, dim] layout -- one PSUM
accumulation group per bank, no transposes anywhere:
    qT,kT = [dim, own-m]  (lhsT=W slice, rhs=xT);  vT likewise
    V'    = [own-m, do]   (lhsT=vT slice, rhs=Wout)
    S^T[j,i] from lhsT=kT-chunk, rhs=qT;  P^T = exp(S^T * scale) * mask
    out[i,do] += lhsT=P^T i-slice, rhs=V'-chunk;  sums[i] via rhs=ones
Softmax uses a fixed max of 0 (logits ~ N(0,1); exp cannot overflow).
Compute dtype bf16, fp32 PSUM accumulation.

K^T/V' shards are AllGathered in 4 stages (own-m quarter each) so cores
start attention on early key chunks while later stages are in flight.
Keys are processed in gathered (rank-major) order -- softmax is
order-invariant over keys; only the masks know true global positions.
"""

from contextlib import ExitStack

import numpy as np
import ml_dtypes

import concourse.bass as bass
from concourse import bacc
import concourse.mybir as mybir
import concourse.tile as tile
from concourse.bass_utils import run_bass_kernel_spmd

BF16 = mybir.dt.bfloat16
F32 = mybir.dt.float32

NCORES = 8
NQ = 4   # query tiles per core
NS = 4   # AllGather stages


def build_nc(N=8192, D=1024):
    A = D // 128          # contraction d-tiles
    R = N // NCORES       # own rows per core
    IT = R // NQ          # query-tile width (256 full)
    IH = IT // 2          # query half-tile = PV output partition (128 full)
    CH = IT // 2          # own-m rows per key chunk (128 full)
    MS = R // NS          # own-m rows per AG stage (= 2*CH)
    DH = min(512, D)      # do-half width for PV outputs
    NDH = D // DH         # number of do-halves (2 full)
    KV_K = D * MS         # kT elems per stage ([D, MS] d-major)
    KV_V = MS * D         # V' elems per stage ([MS, D] row-major)
    KV = KV_K + KV_V
    SCALE = 1.0 / float(np.sqrt(D))

    nc = bacc.Bacc(None, num_devices=NCORES)

    xt_ext = nc.declare_dram_parameter("xt", [D, R], BF16, isOutput=False)
    wqkv_ext = nc.declare_dram_parameter("wqkv", [D, 3 * D], BF16,
                                         isOutput=False)
    wout_ext = nc.declare_dram_parameter("wout", [D, D], BF16, isOutput=False)
    bout_ext = nc.declare_dram_parameter("bout", [1, D], F32, isOutput=False)
    # per-core causal masks: [CH(j), 2(mb parity), 8(rank), IT(i)]
    msk_ext = nc.declare_dram_parameter("msk", [CH, 2, NCORES, IT], BF16,
                                        isOutput=False)
    out_ext = nc.declare_dram_parameter("out", [R, D], F32, isOutput=True)

    kvin = [nc.dram_tensor(f"kvin_{s}", [KV], BF16) for s in range(NS)]
    gath = [
        nc.dram_tensor(f"gath_{s}", [NCORES, KV], BF16, addr_space="Shared")
        for s in range(NS)
    ]

    with ExitStack() as ctx:
        tc = ctx.enter_context(tile.TileContext(nc))
        sb = ctx.enter_context(tc.tile_pool(name="sb", bufs=1))
        ps = ctx.enter_context(tc.tile_pool(name="ps", bufs=1, space="PSUM"))

        # ---- resident SBUF ---------------------------------------------
        xt_sb = sb.tile([128, A, R], BF16, name="xt_sb")
        wq_sb = sb.tile([128, A, D], BF16, name="wq_sb")
        wk_sb = sb.tile([128, A, D], BF16, name="wk_sb")
        wv_sb = sb.tile([128, A, D], BF16, name="wv_sb")
        wo_sb = sb.tile([128, A, D], BF16, name="wo_sb")
        qt_sb = sb.tile([128, A, R], BF16, name="qt_sb")
        msk_sb = sb.tile([CH, 2, NCORES, IT], BF16, name="msk_sb")
        ones_sb = sb.tile([128, 1], BF16, name="ones_sb")
        onerow_sb = sb.tile([1, 128], F32, name="onerow_sb")
        bob_sb = sb.tile([128, D], F32, name="bob_sb")

        nc.vector.memset(ones_sb, 1.0)
        nc.vector.memset(onerow_sb, 1.0)

        def load_t(dst, src_ap):
            nc.sync.dma_start(out=dst,
                              in_=src_ap.rearrange("(a p) n -> p a n", p=128))

        load_t(wk_sb, wqkv_ext[:, D:2 * D])
        for s in range(NS):
            nc.sync.dma_start(
                out=xt_sb[:, :, MS * s:MS * (s + 1)],
                in_=xt_ext[:, MS * s:MS * (s + 1)].rearrange(
                    "(a p) n -> p a n", p=128))
        load_t(wv_sb, wqkv_ext[:, 2 * D:3 * D])
        load_t(wo_sb, wout_ext[:, :])
        load_t(wq_sb, wqkv_ext[:, 0:D])
        nc.sync.dma_start(out=msk_sb, in_=msk_ext[:, :, :, :])

        # broadcast bout across partitions with a step-0 DMA
        bo_src = bout_ext[0:1, :]
        bo_bc = bass.AP(tensor=bo_src.tensor, offset=bo_src.offset,
                        ap=[[0, 128], bo_src.ap[1]])
        nc.sync.dma_start(out=bob_sb, in_=bo_bc)

        # ---- projections, staged so each AllGather launches ASAP --------
        def proj_T(dst_sb, w_sb, c0, c1, d0):
            W = min(512, c1 - c0)
            for m in range(A):
                for h in range((c1 - c0) // W):
                    lo = c0 + W * h
                    acc = ps.tile([128, W], F32, tag="mm", bufs=2,
                                  name="proj_ps")
                    for a in range(A):
                        nc.tensor.matmul(
                            acc,
                            w_sb[:, a, 128 * m:128 * (m + 1)],
                            xt_sb[:, a, lo:lo + W],
                            start=(a == 0), stop=(a == A - 1),
                        )
                    nc.vector.tensor_copy(
                        dst_sb[:, m, d0 + lo - c0:d0 + lo - c0 + W], acc)

        for s in range(NS):
            # per-stage projection tiles (rotate; frees SBUF for prefetch)
            kt_st = sb.tile([128, A, MS], BF16, tag="kt_st", bufs=2,
                            name="kt_st")
            vt_st = sb.tile([128, A, MS], BF16, tag="vt_st", bufs=2,
                            name="vt_st")
            vp_st = sb.tile([CH, MS // CH, D], BF16, tag="vp_st", bufs=2,
                            name="vp_st")
            proj_T(kt_st, wk_sb, MS * s, MS * (s + 1), 0)
            proj_T(vt_st, wv_sb, MS * s, MS * (s + 1), 0)
            # V' = V @ Wout for this stage's rows
            for t in range(MS // CH):
                for h in range(D // DH):
                    acc = ps.tile([CH, DH], F32, tag="mm", bufs=2,
                                  name="vp_ps")
                    for a in range(A):
                        nc.tensor.matmul(
                            acc,
                            vt_st[:, a, CH * t:CH * (t + 1)],
                            wo_sb[:, a, DH * h:DH * (h + 1)],
                            start=(a == 0), stop=(a == A - 1),
                        )
                    nc.vector.tensor_copy(
                        vp_st[:, t, DH * h:DH * (h + 1)], acc)
            kpart = kvin[s][0:KV_K].rearrange("(a p m) -> p a m", p=128, a=A)
            nc.sync.dma_start(out=kpart, in_=kt_st[:, :, :])
            vpart = kvin[s][KV_K:KV].rearrange("(t p d) -> p t d", p=CH,
                                               t=MS // CH)
            for ct in range(MS // CH):
                nc.sync.dma_start(out=vpart[:, ct, :], in_=vp_st[:, ct, :])
            nc.gpsimd.collective_compute(
                "AllGather",
                mybir.AluOpType.bypass,
                replica_groups=[list(range(NCORES))],
                ins=[kvin[s][:]],
                outs=[gath[s][:, :]],
            )

        proj_T(qt_sb, wq_sb, 0, R, 0)

        # ---- attention --------------------------------------------------
        # query tile q: own-m in [IT*q, IT*(q+1)); key chunks (r, mb) with
        # mb in [0, 2q+2) over all 8 ranks, in gathered order.
        for q in range(NQ):
            psO = [ps.tile([IH, DH], F32, tag="oacc", bufs=2 * NDH,
                           name=f"psO{ih}_{dh}")
                   for ih in range(2) for dh in range(NDH)]
            sums = [ps.tile([IH, 1], F32, tag="sums", bufs=2,
                            name=f"sums{ih}") for ih in range(2)]
            first = True
            for st in range(q + 1):
                for r in range(NCORES):
                    ktc = sb.tile([128, A, MS], BF16, tag="ktc", bufs=4,
                                  name="ktc")
                    nc.sync.dma_start(
                        out=ktc,
                        in_=gath[st][r, 0:KV_K].rearrange(
                            "(a p m) -> p a m", p=128, a=A))
                    vpc = sb.tile([CH, MS // CH, D], BF16, tag="vpc", bufs=4,
                                  name="vpc")
                    nc.sync.dma_start(
                        out=vpc,
                        in_=gath[st][r, KV_K:KV].rearrange(
                            "(t p d) -> p t d", p=CH, t=MS // CH))
                    for mloc in range(MS // CH):
                        mb = 2 * st + mloc
                        s_ps = ps.tile([CH, IT], F32, tag="mm", bufs=2,
                                       name="s_ps")
                        for a in range(A):
                            nc.tensor.matmul(
                                s_ps,
                                ktc[:, a, CH * mloc:CH * (mloc + 1)],
                                qt_sb[:, a, IT * q:IT * (q + 1)],
                                start=(a == 0), stop=(a == A - 1),
                            )
                        pt = sb.tile([CH, IT], BF16, tag="pt", bufs=12,
                                     name="pt")
                        nc.scalar.activation(pt, s_ps,
                                             mybir.ActivationFunctionType.Exp,
                                             scale=SCALE)
                        if st == q:  # diagonal chunk: causal mask
                            nc.vector.tensor_mul(pt, pt,
                                                 msk_sb[:, mloc, r, :])
                        lastr = (st == q and r == NCORES - 1
                                 and mloc == MS // CH - 1)
                        for ih in range(2):
                            for dh in range(NDH):
                                nc.tensor.matmul(
                                    psO[ih * NDH + dh],
                                    pt[:, IH * ih:IH * (ih + 1)],
                                    vpc[:, mloc, DH * dh:DH * (dh + 1)],
                                    start=first, stop=lastr)
                            nc.tensor.matmul(sums[ih],
                                             pt[:, IH * ih:IH * (ih + 1)],
                                             ones_sb[:CH, :],
                                             start=first, stop=lastr)
                        first = False

            # epilogue: out = psO * (1/sums) + bout ; store
            for ih in range(2):
                recip = sb.tile([IH, 1], F32, tag="recip", bufs=4,
                                name="recip")
                nc.vector.reciprocal(recip, sums[ih])
                for dh in range(NDH):
                    ot_sb = sb.tile([IH, DH], F32, tag="ot", bufs=4,
                                    name="ot_sb")
                    nc.scalar.activation(ot_sb, psO[ih * NDH + dh],
                                         mybir.ActivationFunctionType.Identity,
                                         scale=recip)
                    nc.vector.tensor_add(ot_sb, ot_sb,
                                         bob_sb[:IH, DH * dh:DH * (dh + 1)])
                    nc.sync.dma_start(
                        out=out_ext[IT * q + IH * ih:IT * q + IH * (ih + 1),
                                    DH * dh:DH * (dh + 1)],
                        in_=ot_sb)

    nc.compile()
    return nc


# ---------------------------------------------------------------------------
# host side
# ---------------------------------------------------------------------------

def make_masks(c, N=8192, D=1024):
    """Masks for core c: msk[x, parity, r, y] = 1 iff key row (own-m index
    CH*parity+x within its diagonal chunk, rank r) is causal for query
    (own-m index IT*q+y of core c): x - y <= -CH*parity - (r > c)."""
    R = N // NCORES
    IT = R // NQ
    CH = IT // 2
    x = np.arange(CH)[:, None]
    y = np.arange(IT)[None, :]
    msk = np.zeros((CH, 2, NCORES, IT), dtype=np.float32)
    for par in range(2):
        for r in range(NCORES):
            lim = -CH * par - (1 if r > c else 0)
            msk[:, par, r, :] = (x - y <= lim).astype(np.float32)
    return msk.astype(ml_dtypes.bfloat16)


_CACHE = {}


def _build(N, D):
    key = (N, D)
    if key not in _CACHE:
        _CACHE[key] = build_nc(N, D)
    return _CACHE[key]


def run(x, Wqkv, Wout, bout, trace=False, N=8192, D=1024):
    nc = _build(N, D)
    bf = ml_dtypes.bfloat16
    wqkv_b = np.ascontiguousarray(Wqkv).astype(bf)
    wout_b = np.ascontiguousarray(Wout).astype(bf)
    bout_r = np.ascontiguousarray(
        np.asarray(bout, dtype=np.float32).reshape(1, D))
    in_maps = []
    for c in range(NCORES):
        xt_c = np.ascontiguousarray(np.asarray(x)[c::NCORES, :].T).astype(bf)
        in_maps.append({
            "xt": xt_c,
            "wqkv": wqkv_b,
            "wout": wout_b,
            "bout": bout_r,
            "msk": make_masks(c, N, D),
        })
    res = run_bass_kernel_spmd(nc, in_maps, list(range(NCORES)), trace=trace)
    out = np.empty((N, D), dtype=np.float32)
    for c in range(NCORES):
        out[c::NCORES, :] = res.results[c]["out"]
    return out, res


def kernel(**inputs):
    out, _ = run(inputs["x"], inputs["Wqkv"], inputs["Wout"], inputs["bout"],
                 trace=False)
    return out

